# revision 11
# baseline (speedup 1.0000x reference)
"""Causal self-attention with RoPE on 8 Trainium2 NeuronCores.

Problem: B=4, T=2048, C=1024, NH=16, D=64. y = proj(attn(rope(qkv(x)))).

Sharding: core = (batch b, head-group hg): 4 batches x 2 groups of 8 heads.
Each core computes its 8 heads' attention for its batch plus the partial
output projection over its 512 head-channels; the host sums the two
partials per batch and adds b_proj.

On-device layout is "transposed" throughout ([feature partitions, token
free-dim]) so no on-chip transposes are needed:
  - qT/kT produced as [d, t] directly from the QKV matmul
  - RoPE rotate_half done with a constant rotation matmul + elementwise
  - scoresT[kv, q] = kT.T-slice @ qT-slice per 128-kv tile
  - softmax denominator via a ones-column appended to V (free on PE)
  - PV gives yT[d, q]; normalization via reciprocal + partition broadcast
  - output projection consumes yT tiles directly as the stationary operand

x/weights stream in as bf16 (halves DMA); attention internals stay f32r;
yT and the output are bf16 (~1e-3 rel err measured on HW).
"""
import math
from contextlib import ExitStack

import numpy as np
import ml_dtypes

import concourse.bass as bass
import concourse.tile as tile
from concourse import bacc, mybir
from concourse.bass_utils import run_bass_kernel_spmd

B, T, C, NH, D = 4, 2048, 1024, 16, 64
P = 128                 # partitions
GN = 512                # token-group size
TG = T // GN            # 4 token groups
KT = C // P             # 8 contraction tiles over C
NCORES = 8
HPC = 8                 # heads per core
f32 = mybir.dt.float32
f32r = mybir.dt.float32r
bf16 = mybir.dt.bfloat16
AF = mybir.ActivationFunctionType
BF16NP = ml_dtypes.bfloat16

_NC_CACHE = None


def _body(ctx, tc, xT, wqkT, wvT, wpT, csT, cstT, rmat, bv, outp):
    nc = tc.nc

    const = ctx.enter_context(tc.tile_pool(name="const", bufs=1))
    resid = ctx.enter_context(tc.tile_pool(name="resid", bufs=1))
    xpool = ctx.enter_context(tc.tile_pool(name="xpool", bufs=2))
    cspool = ctx.enter_context(tc.tile_pool(name="cspool", bufs=2))
    rawp = ctx.enter_context(tc.tile_pool(name="rawp", bufs=2))
    tmpp = ctx.enter_context(tc.tile_pool(name="tmpp", bufs=2))
    attp = ctx.enter_context(tc.tile_pool(name="attp", bufs=3))
    bcp = ctx.enter_context(tc.tile_pool(name="bcp", bufs=2))
    rcp = ctx.enter_context(tc.tile_pool(name="rcp", bufs=2))
    outsb = ctx.enter_context(tc.tile_pool(name="outsb", bufs=2))
    psmm = ctx.enter_context(tc.tile_pool(name="psmm", bufs=2, space="PSUM"))
    pssc = ctx.enter_context(tc.tile_pool(name="pssc", bufs=2, space="PSUM"))
    psy = ctx.enter_context(tc.tile_pool(name="psy", bufs=2, space="PSUM"))

    # ---- resident tensors; DMA issue order == startup priority ----
    # wqk + x(g=0) gate the first matmul; everything else can trail.
    wqk_sb = const.tile([P, KT * 1024], bf16, tag="wqk", name="wqk_sb")
    nc.sync.dma_start(wqk_sb[:].rearrange("p (k f) -> p k f", k=KT),
                      wqkT.rearrange("(k p) f -> p k f", p=P))

    x_g = []
    for g in range(TG):
        x_ = xpool.tile([P, KT * GN], bf16, tag="xt", name=f"xt{g}")
        if g == 0:
            nc.sync.dma_start(
                x_[:].rearrange("p (k t) -> p k t", k=KT),
                xT[:, 0:GN].rearrange("(k p) t -> p k t", p=P))
        x_g.append(x_)

    # consts: cols [0:128) dmask | [128:136) bqk; rmat separate (f32r)
    consts_t = const.tile([P, 136], f32, tag="consts", name="consts_t")
    nc.sync.dma_start(consts_t[:], cstT[:])
    dmask_t = consts_t[:, 0:128]
    bqk_t = consts_t[:, 128:136]
    rmat_t = const.tile([P, P], f32r, tag="rmat", name="rmat_t")
    nc.sync.dma_start(rmat_t[:], rmat[:])

    cs_g = []
    for g in range(TG):
        cs_ = cspool.tile([P, 2 * GN], f32, tag="cs", name=f"cs{g}")
        if g == 0:
            nc.sync.dma_start(
                cs_[:],
                csT.rearrange("p (two t) -> p two t", two=2)[:, :, 0:GN])
        cs_g.append(cs_)

    bv_t = const.tile([1, 512], f32, tag="bv", name="bv_t")
    nc.sync.dma_start(bv_t[:], bv[:])
    bvb_t = const.tile([P, 512], f32, tag="bvb", name="bvb_t")
    nc.gpsimd.partition_broadcast(bvb_t[:], bv_t[:])
    bvb4 = bvb_t[:].rearrange("p (h e) -> p h e", h=HPC)

    wv_sb = const.tile([P, KT * 512], bf16, tag="wv", name="wv_sb")
    wp_sb = const.tile([P, 4 * 1024], bf16, tag="wp", name="wp_sb")

    kT_t = []
    for p in range(4):
        k_ = resid.tile([P, T], f32r, tag=f"kT{p}", name=f"kT{p}")
        kT_t.append(k_)
    # vplus layout: [128 tok, tt(16) x head(8) x (64 d + 1 ones)]
    vplus = resid.tile([P, 16 * HPC * 65], f32r, tag="vplus", name="vplus")
    vp4 = vplus[:].rearrange("p (t h e) -> p t h e", t=16, h=HPC)
    # ones columns via strided memset (1.0f bit pattern through an f32 view)
    nc.gpsimd.memset(vp4[:, :, :, 64:65].bitcast(f32), 1.0)
    qT_g = []
    for p in range(4):
        q_ = resid.tile([P, GN], f32r, tag=f"qT{p}", name=f"qT{p}")
        qT_g.append(q_)
    yT_g = [resid.tile([P, GN], bf16, tag=f"yT{p}", name=f"yT{p}_t")
            for p in range(4)]

    def _proj(g_):
        # output projection for group g_ (partial over 512 channels).
        # Emitted between group g_+1's QKV phase and its attention: by
        # then the slowest pair's normalization has finished, so the
        # in-order PE queue doesn't stall on it.
        for tt in range(4):
            for n in range(2):
                o_ps = psmm.tile([P, GN], f32, tag="mm",
                                 name=f"ops{g_}_{tt}_{n}")
                for p in range(4):
                    nc.tensor.matmul(o_ps[:], yT_g[p][:, tt * P:(tt + 1) * P],
                                     wp_sb[:, p * 1024 + n * GN:
                                           p * 1024 + (n + 1) * GN],
                                     start=(p == 0), stop=(p == 3))
                o_sb = outsb.tile([P, GN], bf16, tag="osb",
                                  name=f"osb{g_}_{tt}_{n}")
                nc.vector.tensor_copy(o_sb[:], o_ps[:])
                nc.sync.dma_start(
                    outp[g_ * GN + tt * P: g_ * GN + (tt + 1) * P,
                         n * GN:(n + 1) * GN], o_sb[:])

    for g in range(TG):
        gsl = slice(g * GN, (g + 1) * GN)
        # ---- per-group loads (prefetched one group ahead) ----
        if g + 1 < TG:
            nsl = slice((g + 1) * GN, (g + 2) * GN)
            nc.sync.dma_start(
                x_g[g + 1][:].rearrange("p (k t) -> p k t", k=KT),
                xT[:, nsl].rearrange("(k p) t -> p k t", p=P))
            nc.sync.dma_start(
                cs_g[g + 1][:],
                csT.rearrange("p (two t) -> p two t", two=2)[:, :, nsl])
        x_t = x_g[g]
        cos_t = cs_g[g][:, 0:GN]
        sin_t = cs_g[g][:, GN:2 * GN]

        # ---- QKV projection for q/k feats (8 tiles of 128 feats) + RoPE ----
        # The RoPE rotation matmul for feat f is emitted after feat f+1's
        # QKV matmuls so the PE never waits on the bias-add vector op.
        def _rot(f, raw):
            rot_ps = psmm.tile([P, GN], f32, tag="mm", name=f"rotps{g}_{f}")
            nc.tensor.matmul(rot_ps[:], rmat_t[:], raw[:], start=True, stop=True)
            tmp = tmpp.tile([P, GN], f32, tag="tmp", name=f"tmp{g}_{f}")
            nc.vector.tensor_mul(tmp[:], rot_ps[:], sin_t)
            dst = qT_g[f][:] if f < 4 else kT_t[f - 4][:, gsl]
            nc.vector.tensor_mul(dst, raw[:], cos_t)
            nc.vector.tensor_add(dst, dst, tmp[:])

        pending_rot = None
        for f in range(8):
            mm_ps = psmm.tile([P, GN], f32, tag="mm", name=f"qkps{g}_{f}")
            for k in range(KT):
                nc.tensor.matmul(mm_ps[:],
                                 wqk_sb[:, k * 1024 + f * P:
                                        k * 1024 + (f + 1) * P],
                                 x_t[:, k * GN:(k + 1) * GN],
                                 start=(k == 0), stop=(k == KT - 1))
            raw = rawp.tile([P, GN], f32r, tag="raw", name=f"raw{g}_{f}")
            nc.vector.tensor_scalar_add(raw[:], mm_ps[:], bqk_t[:, f:f + 1])
            if pending_rot is not None:
                _rot(*pending_rot)
            pending_rot = (f, raw)
        _rot(*pending_rot)

        # ---- V projection into vplus (+ b_v via broadcast add) ----
        if g == 0:
            nc.sync.dma_start(
                wv_sb[:].rearrange("p (k f) -> p k f", k=KT),
                wvT.rearrange("(k p) f -> p k f", p=P))
            nc.sync.dma_start(
                wp_sb[:].rearrange("p (k f) -> p k f", k=4),
                wpT.rearrange("(k p) f -> p k f", p=P))
        for tt in range(4):
            ttg = g * 4 + tt
            v_ps = psmm.tile([P, GN], f32, tag="mm", name=f"vps{g}_{tt}")
            for k in range(KT):
                nc.tensor.matmul(v_ps[:],
                                 x_t[:, k * GN + tt * P:k * GN + (tt + 1) * P],
                                 wv_sb[:, k * 512:(k + 1) * 512],
                                 start=(k == 0), stop=(k == KT - 1))
            nc.vector.tensor_add(vp4[:, ttg, :, 0:64],
                                 v_ps[:].rearrange("p (h e) -> p h e", h=HPC),
                                 bvb4)

        # ---- previous group's output projection (PE filler) ----
        if g > 0:
            _proj(g - 1)

        # ---- attention: head pairs, even/odd fused in one 2-bank PSUM ----
        njt = 4 * g + 4                      # kv tiles for this q-group

        def _finish_norm(p_, rc):
            # broadcasts + final normalize muls for pair p_; emitted one
            # pair late so the reciprocal latency hides behind the next
            # pair's attention instead of stalling the in-order queues.
            bcb = bcp.tile([P, GN], f32, tag="bcb", name=f"bcb{g}_{p_}")
            nc.gpsimd.partition_broadcast(bcb[0:64, :], rc[:, 0:GN])
            nc.vector.tensor_mul(yT_g[p_][0:64, :], yT_g[p_][0:64, :],
                                 bcb[0:64, :])
            bcb2 = bcp.tile([P, GN], f32, tag="bcb", name=f"bcb2{g}_{p_}")
            nc.gpsimd.partition_broadcast(bcb2[0:64, :], rc[:, GN:2 * GN])
            nc.sync.dma_start(bcb2[64:128, :], bcb2[0:64, :])
            nc.vector.tensor_mul(yT_g[p_][64:128, :],
                                 yT_g[p_][64:128, :], bcb2[64:128, :])

        pending_norm = None
        for p in range(4):
            yps = [None, None]
            for s in range(2):
                yps[s] = psy.tile([65, GN], f32, tag="y", name=f"yps{g}_{p}_{s}")
            # software-pipelined: QK/exp for tile j+1 are issued BEFORE the
            # PV of tile j so the in-order PE queue never stalls on exp.
            prev_a2 = None

            # diagonal tiles first (r=0 leads with a full-width start);
            # later diagonals are narrowed to their causally live columns.
            jorder = list(range(4 * g, njt)) + list(range(0, 4 * g))

            def _pv(ji_, a2_):
                j_ = jorder[ji_]
                c0_ = max(j_ - 4 * g, 0) * P
                for s in range(2):
                    h = 2 * p + s
                    nc.tensor.matmul(yps[s][:, c0_:GN],
                                     vp4[:, j_, h, :],
                                     a2_[:, s * GN + c0_:(s + 1) * GN],
                                     start=(ji_ == 0), stop=(ji_ == njt - 1),
                                     skip_group_check=True)

            for ji in range(njt):
                j = jorder[ji]
                r = j - 4 * g                # >=0 on diagonal tiles
                c0 = max(r, 0) * P           # first valid q column
                sc2 = pssc.tile([P, 2 * GN], f32, tag="sc",
                                name=f"sc{g}_{p}_{j}")
                for s in range(2):
                    hb = s * 64
                    nc.tensor.matmul(
                        sc2[:, s * GN + c0:(s + 1) * GN],
                        kT_t[p][hb:hb + 64, j * P:(j + 1) * P],
                        qT_g[p][hb:hb + 64, c0:GN],
                        start=True, stop=True)
                a2 = attp.tile([P, 2 * GN], f32r, tag="att",
                               name=f"att{g}_{p}_{j}")
                sc2v = sc2[:].rearrange("p (s q) -> p s q", s=2)
                a2v = a2[:].rearrange("p (s q) -> p s q", s=2)
                nc.scalar.activation(a2v[:, :, c0:GN], sc2v[:, :, c0:GN],
                                     AF.Exp, scale=1.0 / math.sqrt(D))
                if r >= 0:
                    nc.vector.tensor_mul(a2[:, c0:c0 + P],
                                         a2[:, c0:c0 + P], dmask_t)
                    nc.vector.tensor_mul(a2[:, GN + c0:GN + c0 + P],
                                         a2[:, GN + c0:GN + c0 + P], dmask_t)
                if prev_a2 is not None:
                    _pv(ji - 1, prev_a2)
                prev_a2 = a2
            _pv(njt - 1, prev_a2)
            if pending_norm is not None:
                _finish_norm(*pending_norm)
            # tail: evacuate y-body fast (frees the yps banks so the next
            # pair's PV can start), then reciprocal of both rowsum rows in
            # one on-chip op (no DMA roundtrips).
            for s in range(2):
                hb = s * 64
                nc.vector.tensor_copy(yT_g[p][hb:hb + 64, :], yps[s][0:64, :])
            rs = rcp.tile([1, 2 * GN], f32, tag="rs", name=f"rs{g}_{p}")
            nc.vector.tensor_copy(rs[:, 0:GN], yps[0][64:65, :])
            nc.vector.tensor_copy(rs[:, GN:2 * GN], yps[1][64:65, :])
            rc = rcp.tile([1, 2 * GN], f32, tag="rc", name=f"rc{g}_{p}")
            nc.vector.reciprocal(rc[:], rs[:])
            pending_norm = (p, rc)
        _finish_norm(*pending_norm)
    _proj(TG - 1)


def build_nc():
    nc = bacc.Bacc("TRN2", target_bir_lowering=False, debug=False,
                   num_devices=NCORES)
    xT = nc.dram_tensor("xT", [C, T], bf16, kind="ExternalInput").ap()
    wqkT = nc.dram_tensor("wqkT", [C, 1024], bf16, kind="ExternalInput").ap()
    wvT = nc.dram_tensor("wvT", [C, 512], bf16, kind="ExternalInput").ap()
    wpT = nc.dram_tensor("wpT", [512, 1024], bf16, kind="ExternalInput").ap()
    csT = nc.dram_tensor("csT", [P, 2 * T], f32, kind="ExternalInput").ap()
    cstT = nc.dram_tensor("cstT", [P, 136], f32, kind="ExternalInput").ap()
    rmat = nc.dram_tensor("rmat", [P, P], f32r, kind="ExternalInput").ap()
    bv = nc.dram_tensor("bv", [1, 512], f32, kind="ExternalInput").ap()
    outp = nc.dram_tensor("outp", [T, C], bf16, kind="ExternalOutput").ap()
    with tile.TileContext(nc) as tc, \
            nc.allow_low_precision(reason="bf16/f32r matmul operands"):
        with ExitStack() as ctx:
            _body(ctx, tc, xT, wqkT, wvT, wpT, csT, cstT, rmat, bv, outp)
    nc.compile()
    return nc


def _host_inputs(x, w_attn, b_attn, w_proj, cos, sin):
    """Build the 8 per-core input dicts."""
    # rotation matrix: ROT @ q == rotate_half(q) in [d] space
    rot = np.zeros((D, D), np.float32)
    for d_ in range(32):
        rot[d_, d_ + 32] = -1.0
        rot[d_ + 32, d_] = 1.0
    rmat = np.zeros((P, P), np.float32)
    rmat[0:D, 0:D] = rot.T
    rmat[D:P, D:P] = rot.T
    dmask = np.triu(np.ones((P, P), np.float32))
    cosT2 = np.ascontiguousarray(
        np.concatenate([cos[0].T, cos[0].T], axis=0))      # [128, T]
    sinT2 = np.ascontiguousarray(np.concatenate([sin[0].T, sin[0].T], axis=0))
    csT = np.ascontiguousarray(np.concatenate([cosT2, sinT2], axis=1))

    in_maps = []
    for core in range(NCORES):
        b = core // 2
        hg = core % 2
        h0 = hg * HPC
        qrows = slice(h0 * D, (h0 + HPC) * D)              # 512 rows
        krows = slice(C + h0 * D, C + (h0 + HPC) * D)
        vrows = slice(2 * C + h0 * D, 2 * C + (h0 + HPC) * D)
        wqk = np.concatenate([w_attn[qrows], w_attn[krows]], axis=0)  # [1024, C]
        bqk_np = np.concatenate([b_attn[qrows], b_attn[krows]])       # [1024]
        cst = np.concatenate([dmask,
                              bqk_np.reshape(8, P).T.astype(np.float32)],
                             axis=1)                                  # [128, 136]
        in_maps.append({
            "xT": np.ascontiguousarray(x[b].T).astype(BF16NP),         # [C, T]
            "wqkT": np.ascontiguousarray(wqk.T).astype(BF16NP),        # [C, 1024]
            "wvT": np.ascontiguousarray(w_attn[vrows].T).astype(BF16NP),
            "wpT": np.ascontiguousarray(
                w_proj[:, h0 * D:(h0 + HPC) * D].T).astype(BF16NP),
            "csT": csT,
            "cstT": np.ascontiguousarray(cst),
            "rmat": rmat,
            "bv": np.ascontiguousarray(b_attn[vrows].reshape(1, 512)
                                       .astype(np.float32)),
        })
    return in_maps


def kernel(x, w_attn, b_attn, w_proj, b_proj, cos, sin):
    global _NC_CACHE
    x = np.asarray(x, np.float32)
    w_attn = np.asarray(w_attn, np.float32)
    b_attn = np.asarray(b_attn, np.float32)
    w_proj = np.asarray(w_proj, np.float32)
    b_proj = np.asarray(b_proj, np.float32)
    cos = np.asarray(cos, np.float32)
    sin = np.asarray(sin, np.float32)

    if _NC_CACHE is None:
        _NC_CACHE = build_nc()
    nc = _NC_CACHE
    in_maps = _host_inputs(x, w_attn, b_attn, w_proj, cos, sin)
    res = run_bass_kernel_spmd(nc, in_maps, core_ids=list(range(NCORES)))
    parts = [np.asarray(res.results[i]["outp"], np.float32)
             for i in range(NCORES)]
    out = np.empty((B, T, C), np.float32)
    for b in range(B):
        out[b] = parts[2 * b] + parts[2 * b + 1] + b_proj
    return out


# revision 18
# speedup vs baseline: 1.3067x; 1.3067x over previous
"""Causal self-attention with RoPE on 8 Trainium2 NeuronCores.

Problem: B=4, T=2048, C=1024, NH=16, D=64. y = proj(attn(rope(qkv(x)))).

Sharding: core = (batch b, head-group hg): 4 batches x 2 groups of 8 heads.
Each core computes its 8 heads' attention for its batch plus the partial
output projection over its 512 head-channels; the host sums the two
partials per batch and adds b_proj.

On-device layout is "transposed" throughout ([feature partitions, token
free-dim]) so no on-chip transposes are needed:
  - qT/kT produced as [d, t] directly from the QKV matmul
  - RoPE rotate_half done with a constant rotation matmul + elementwise
  - scoresT[kv, q] = kT.T-slice @ qT-slice per 128-kv tile
  - softmax denominator via a ones-column appended to V (free on PE)
  - PV gives yT[d, q]; normalization via reciprocal + partition broadcast
  - output projection consumes yT tiles directly as the stationary operand

x/weights stream in as bf16 (halves DMA); attention internals stay f32r;
yT and the output are bf16 (~1e-3 rel err measured on HW).
"""
import math
from contextlib import ExitStack

import numpy as np
import ml_dtypes

import concourse.bass as bass
import concourse.tile as tile
from concourse import bacc, mybir
from concourse.bass_utils import run_bass_kernel_spmd

B, T, C, NH, D = 4, 2048, 1024, 16, 64
P = 128                 # partitions
GN = 512                # token-group size
TG = T // GN            # 4 token groups
KT = C // P             # 8 contraction tiles over C
NCORES = 8
HPC = 8                 # heads per core
f32 = mybir.dt.float32
f32r = mybir.dt.float32r
bf16 = mybir.dt.bfloat16
AF = mybir.ActivationFunctionType
BF16NP = ml_dtypes.bfloat16

_NC_CACHE = None


def _body(ctx, tc, xT, wqkT, wvT, wpT, csT, cstT, rmat, bv, outp):
    nc = tc.nc

    const = ctx.enter_context(tc.tile_pool(name="const", bufs=1))
    resid = ctx.enter_context(tc.tile_pool(name="resid", bufs=1))
    xpool = ctx.enter_context(tc.tile_pool(name="xpool", bufs=2))
    cspool = ctx.enter_context(tc.tile_pool(name="cspool", bufs=2))
    rawp = ctx.enter_context(tc.tile_pool(name="rawp", bufs=2))
    tmpp = ctx.enter_context(tc.tile_pool(name="tmpp", bufs=2))
    attp = ctx.enter_context(tc.tile_pool(name="attp", bufs=3))
    bcp = ctx.enter_context(tc.tile_pool(name="bcp", bufs=2))
    rcp = ctx.enter_context(tc.tile_pool(name="rcp", bufs=2))
    rsp = ctx.enter_context(tc.tile_pool(name="rsp", bufs=2))
    outsb = ctx.enter_context(tc.tile_pool(name="outsb", bufs=2))
    psmm = ctx.enter_context(tc.tile_pool(name="psmm", bufs=2, space="PSUM"))
    pssc = ctx.enter_context(tc.tile_pool(name="pssc", bufs=2, space="PSUM"))
    psy = ctx.enter_context(tc.tile_pool(name="psy", bufs=2, space="PSUM"))

    # ---- resident tensors; DMA issue order == startup priority ----
    # wqk + x(g=0) gate the first matmul; split in halves so the first
    # accumulation chain starts after ~1.5MB instead of 3MB.
    wqk_sb = [const.tile([P, 4 * 1024], bf16, tag=f"wqk{h}", name=f"wqk_sb{h}")
              for h in range(2)]
    x_g = [[xpool.tile([P, 4 * GN], bf16, tag=f"xt{h}", name=f"xt{g}_{h}")
            for h in range(2)] for g in range(TG)]
    for h in range(2):
        nc.sync.dma_start(
            wqk_sb[h][:].rearrange("p (k f) -> p k f", k=4),
            wqkT[h * 512:(h + 1) * 512, :].rearrange("(k p) f -> p k f", p=P))
        nc.sync.dma_start(
            x_g[0][h][:].rearrange("p (k t) -> p k t", k=4),
            xT[h * 512:(h + 1) * 512, 0:GN].rearrange("(k p) t -> p k t", p=P))

    def wqk_ap(k, fsl):
        return wqk_sb[k // 4][:, (k % 4) * 1024 + fsl.start:
                              (k % 4) * 1024 + fsl.stop]

    def x_ap(g_, k, tsl):
        return x_g[g_][k // 4][:, (k % 4) * GN + tsl.start:
                               (k % 4) * GN + tsl.stop]

    # consts: cols [0:128) dmask | [128:136) bqk; rmat separate (f32r)
    consts_t = const.tile([P, 136], f32, tag="consts", name="consts_t")
    nc.sync.dma_start(consts_t[:], cstT[:])
    dmask_t = consts_t[:, 0:128]
    bqk_t = consts_t[:, 128:136]
    rmat_t = const.tile([P, P], f32r, tag="rmat", name="rmat_t")
    nc.sync.dma_start(rmat_t[:], rmat[:])

    cs_g = []
    for g in range(TG):
        cs_ = cspool.tile([P, 2 * GN], f32, tag="cs", name=f"cs{g}")
        if g == 0:
            nc.sync.dma_start(
                cs_[:],
                csT.rearrange("p (two t) -> p two t", two=2)[:, :, 0:GN])
        cs_g.append(cs_)

    bv_t = const.tile([1, 512], f32, tag="bv", name="bv_t")
    nc.sync.dma_start(bv_t[:], bv[:])
    bvb_t = const.tile([P, 512], f32, tag="bvb", name="bvb_t")
    nc.gpsimd.partition_broadcast(bvb_t[:], bv_t[:])
    bvb4 = bvb_t[:].rearrange("p (h e) -> p h e", h=HPC)

    wv_sb = const.tile([P, KT * 512], bf16, tag="wv", name="wv_sb")
    wp_sb = const.tile([P, 4 * 1024], bf16, tag="wp", name="wp_sb")

    kT_t = []
    for p in range(4):
        k_ = resid.tile([P, T], f32r, tag=f"kT{p}", name=f"kT{p}")
        kT_t.append(k_)
    # vplus layout: [128 tok, tt(16) x head(8) x (64 d + 1 ones)]
    vplus = resid.tile([P, 16 * HPC * 65], f32r, tag="vplus", name="vplus")
    vp4 = vplus[:].rearrange("p (t h e) -> p t h e", t=16, h=HPC)
    # ones columns via strided memset (1.0f bit pattern through an f32 view)
    nc.gpsimd.memset(vp4[:, :, :, 64:65].bitcast(f32), 1.0)
    qT_g = []
    for p in range(4):
        q_ = resid.tile([P, GN], f32r, tag=f"qT{p}", name=f"qT{p}")
        qT_g.append(q_)
    yT_g = [resid.tile([P, GN], bf16, tag=f"yT{p}", name=f"yT{p}_t")
            for p in range(4)]

    def _proj(g_):
        # output projection for group g_ (partial over 512 channels).
        # Emitted between group g_+1's QKV phase and its attention: by
        # then the slowest pair's normalization has finished, so the
        # in-order PE queue doesn't stall on it.
        for tt in range(4):
            for n in range(2):
                o_ps = psmm.tile([P, GN], f32, tag="mm",
                                 name=f"ops{g_}_{tt}_{n}")
                for p in range(4):
                    nc.tensor.matmul(o_ps[:], yT_g[p][:, tt * P:(tt + 1) * P],
                                     wp_sb[:, p * 1024 + n * GN:
                                           p * 1024 + (n + 1) * GN],
                                     start=(p == 0), stop=(p == 3))
                o_sb = outsb.tile([P, GN], bf16, tag="osb",
                                  name=f"osb{g_}_{tt}_{n}")
                nc.vector.tensor_copy(o_sb[:], o_ps[:])
                nc.sync.dma_start(
                    outp[g_ * GN + tt * P: g_ * GN + (tt + 1) * P,
                         n * GN:(n + 1) * GN], o_sb[:])

    for g in range(TG):
        gsl = slice(g * GN, (g + 1) * GN)
        # ---- per-group loads (prefetched one group ahead) ----
        if g + 1 < TG:
            nsl = slice((g + 1) * GN, (g + 2) * GN)
            for h in range(2):
                nc.sync.dma_start(
                    x_g[g + 1][h][:].rearrange("p (k t) -> p k t", k=4),
                    xT[h * 512:(h + 1) * 512, nsl]
                    .rearrange("(k p) t -> p k t", p=P))
            nc.sync.dma_start(
                cs_g[g + 1][:],
                csT.rearrange("p (two t) -> p two t", two=2)[:, :, nsl])
        cos_t = cs_g[g][:, 0:GN]
        sin_t = cs_g[g][:, GN:2 * GN]

        # ---- QKV projection for q/k feats (8 tiles of 128 feats) + RoPE ----
        # The RoPE rotation matmul for feat f is emitted after feat f+1's
        # QKV matmuls so the PE never waits on the bias-add vector op.
        def _rot(f, raw):
            rot_ps = psmm.tile([P, GN], f32, tag="mm", name=f"rotps{g}_{f}")
            nc.tensor.matmul(rot_ps[:], rmat_t[:], raw[:], start=True, stop=True)
            tmp = tmpp.tile([P, GN], f32, tag="tmp", name=f"tmp{g}_{f}")
            nc.vector.tensor_mul(tmp[:], rot_ps[:], sin_t)
            dst = qT_g[f][:] if f < 4 else kT_t[f - 4][:, gsl]
            nc.vector.tensor_mul(dst, raw[:], cos_t)
            nc.vector.tensor_add(dst, dst, tmp[:])

        pending_rot = None
        for f in range(8):
            mm_ps = psmm.tile([P, GN], f32, tag="mm", name=f"qkps{g}_{f}")
            for k in range(KT):
                nc.tensor.matmul(mm_ps[:],
                                 wqk_ap(k, slice(f * P, (f + 1) * P)),
                                 x_ap(g, k, slice(0, GN)),
                                 start=(k == 0), stop=(k == KT - 1))
            raw = rawp.tile([P, GN], f32r, tag="raw", name=f"raw{g}_{f}")
            nc.vector.tensor_scalar_add(raw[:], mm_ps[:], bqk_t[:, f:f + 1])
            if pending_rot is not None:
                _rot(*pending_rot)
            pending_rot = (f, raw)
        _rot(*pending_rot)

        # ---- V projection into vplus (+ b_v via broadcast add) ----
        if g == 0:
            nc.sync.dma_start(
                wv_sb[:].rearrange("p (k f) -> p k f", k=KT),
                wvT.rearrange("(k p) f -> p k f", p=P))
            nc.sync.dma_start(
                wp_sb[:].rearrange("p (k f) -> p k f", k=4),
                wpT.rearrange("(k p) f -> p k f", p=P))
        for tt in range(4):
            ttg = g * 4 + tt
            v_ps = psmm.tile([P, GN], f32, tag="mm", name=f"vps{g}_{tt}")
            for k in range(KT):
                nc.tensor.matmul(v_ps[:],
                                 x_ap(g, k, slice(tt * P, (tt + 1) * P)),
                                 wv_sb[:, k * 512:(k + 1) * 512],
                                 start=(k == 0), stop=(k == KT - 1))
            nc.vector.tensor_add(vp4[:, ttg, :, 0:64],
                                 v_ps[:].rearrange("p (h e) -> p h e", h=HPC),
                                 bvb4)

        # ---- previous group's output projection (PE filler) ----
        if g > 0:
            _proj(g - 1)

        # ---- attention: head pairs, even/odd fused in one 2-bank PSUM ----
        njt = 4 * g + 4                      # kv tiles for this q-group

        def _finish_norm(p_, rcrows):
            # broadcasts + final normalize muls for pair p_; emitted one
            # pair late so the reciprocal DMA roundtrip latency hides
            # behind the next pair's attention instead of stalling the
            # in-order queues. NB: a [1,1024] single-partition reciprocal
            # costs 6.5us on DVE — the [128,8] DMA transpose is 40x faster.
            bcb = bcp.tile([P, GN], f32, tag="bcb", name=f"bcb{g}_{p_}")
            nc.gpsimd.partition_broadcast(bcb[0:64, :], rcrows[0][:])
            nc.vector.tensor_mul(yT_g[p_][0:64, :], yT_g[p_][0:64, :],
                                 bcb[0:64, :])
            bcb2 = bcp.tile([P, GN], f32, tag="bcb", name=f"bcb2{g}_{p_}")
            nc.gpsimd.partition_broadcast(bcb2[0:64, :], rcrows[1][:])
            nc.sync.dma_start(bcb2[64:128, :], bcb2[0:64, :])
            nc.vector.tensor_mul(yT_g[p_][64:128, :],
                                 yT_g[p_][64:128, :], bcb2[64:128, :])

        pending_norm = None
        for p in range(4):
            yps = [None, None]
            for s in range(2):
                yps[s] = psy.tile([65, GN], f32, tag="y", name=f"yps{g}_{p}_{s}")
            # software-pipelined: QK/exp for tile j+1 are issued BEFORE the
            # PV of tile j so the in-order PE queue never stalls on exp.
            prev_a2 = None

            # diagonal tiles first (r=0 leads with a full-width start);
            # later diagonals are narrowed to their causally live columns.
            jorder = list(range(4 * g, njt)) + list(range(0, 4 * g))

            def _pv(ji_, a2_):
                j_ = jorder[ji_]
                c0_ = max(j_ - 4 * g, 0) * P
                for s in range(2):
                    h = 2 * p + s
                    nc.tensor.matmul(yps[s][:, c0_:GN],
                                     vp4[:, j_, h, :],
                                     a2_[:, s * GN + c0_:(s + 1) * GN],
                                     start=(ji_ == 0), stop=(ji_ == njt - 1),
                                     skip_group_check=True)

            for ji in range(njt):
                j = jorder[ji]
                r = j - 4 * g                # >=0 on diagonal tiles
                c0 = max(r, 0) * P           # first valid q column
                sc2 = pssc.tile([P, 2 * GN], f32, tag="sc",
                                name=f"sc{g}_{p}_{j}")
                for s in range(2):
                    hb = s * 64
                    nc.tensor.matmul(
                        sc2[:, s * GN + c0:(s + 1) * GN],
                        kT_t[p][hb:hb + 64, j * P:(j + 1) * P],
                        qT_g[p][hb:hb + 64, c0:GN],
                        start=True, stop=True)
                a2 = attp.tile([P, 2 * GN], f32r, tag="att",
                               name=f"att{g}_{p}_{j}")
                sc2v = sc2[:].rearrange("p (s q) -> p s q", s=2)
                a2v = a2[:].rearrange("p (s q) -> p s q", s=2)
                nc.scalar.activation(a2v[:, :, c0:GN], sc2v[:, :, c0:GN],
                                     AF.Exp, scale=1.0 / math.sqrt(D))
                if r >= 0:
                    nc.vector.tensor_mul(a2[:, c0:c0 + P],
                                         a2[:, c0:c0 + P], dmask_t)
                    nc.vector.tensor_mul(a2[:, GN + c0:GN + c0 + P],
                                         a2[:, GN + c0:GN + c0 + P], dmask_t)
                if prev_a2 is not None:
                    _pv(ji - 1, prev_a2)
                prev_a2 = a2
            _pv(njt - 1, prev_a2)
            if pending_norm is not None:
                _finish_norm(*pending_norm)
            # tail: evacuate y-body fast (frees the yps banks so the next
            # pair's PV can start), then reciprocal of both rowsum rows in
            # one on-chip op (no DMA roundtrips).
            rs_p = rsp.tile([P, 8], f32, tag="rs", name=f"rs{g}_{p}")
            for s in range(2):
                hb = s * 64
                nc.vector.tensor_copy(yT_g[p][hb:hb + 64, :], yps[s][0:64, :])
                rrow = rcp.tile([1, GN], f32, tag="rrow", name=f"rrow{g}_{p}_{s}")
                nc.vector.tensor_copy(rrow[:], yps[s][64:65, :])
                nc.sync.dma_start(rs_p[:, s * 4:(s + 1) * 4], rrow[:])
            rc_p = rsp.tile([P, 8], f32, tag="rc", name=f"rcp{g}_{p}")
            nc.vector.reciprocal(rc_p[:], rs_p[:])
            rcrows = []
            for s in range(2):
                rcrow = rcp.tile([1, GN], f32, tag="rcrow", bufs=2,
                                 name=f"rcrow{g}_{p}_{s}")
                nc.sync.dma_start(rcrow[:], rc_p[:, s * 4:(s + 1) * 4])
                rcrows.append(rcrow)
            pending_norm = (p, rcrows)
        _finish_norm(*pending_norm)
    _proj(TG - 1)


def build_nc():
    nc = bacc.Bacc("TRN2", target_bir_lowering=False, debug=False,
                   num_devices=NCORES)
    xT = nc.dram_tensor("xT", [C, T], bf16, kind="ExternalInput").ap()
    wqkT = nc.dram_tensor("wqkT", [C, 1024], bf16, kind="ExternalInput").ap()
    wvT = nc.dram_tensor("wvT", [C, 512], bf16, kind="ExternalInput").ap()
    wpT = nc.dram_tensor("wpT", [512, 1024], bf16, kind="ExternalInput").ap()
    csT = nc.dram_tensor("csT", [P, 2 * T], f32, kind="ExternalInput").ap()
    cstT = nc.dram_tensor("cstT", [P, 136], f32, kind="ExternalInput").ap()
    rmat = nc.dram_tensor("rmat", [P, P], f32r, kind="ExternalInput").ap()
    bv = nc.dram_tensor("bv", [1, 512], f32, kind="ExternalInput").ap()
    outp = nc.dram_tensor("outp", [T, C], bf16, kind="ExternalOutput").ap()
    with tile.TileContext(nc) as tc, \
            nc.allow_low_precision(reason="bf16/f32r matmul operands"):
        with ExitStack() as ctx:
            _body(ctx, tc, xT, wqkT, wvT, wpT, csT, cstT, rmat, bv, outp)
    nc.compile()
    return nc


def _host_inputs(x, w_attn, b_attn, w_proj, cos, sin):
    """Build the 8 per-core input dicts."""
    # rotation matrix: ROT @ q == rotate_half(q) in [d] space
    rot = np.zeros((D, D), np.float32)
    for d_ in range(32):
        rot[d_, d_ + 32] = -1.0
        rot[d_ + 32, d_] = 1.0
    rmat = np.zeros((P, P), np.float32)
    rmat[0:D, 0:D] = rot.T
    rmat[D:P, D:P] = rot.T
    dmask = np.triu(np.ones((P, P), np.float32))
    cosT2 = np.ascontiguousarray(
        np.concatenate([cos[0].T, cos[0].T], axis=0))      # [128, T]
    sinT2 = np.ascontiguousarray(np.concatenate([sin[0].T, sin[0].T], axis=0))
    csT = np.ascontiguousarray(np.concatenate([cosT2, sinT2], axis=1))

    in_maps = []
    for core in range(NCORES):
        b = core // 2
        hg = core % 2
        h0 = hg * HPC
        qrows = slice(h0 * D, (h0 + HPC) * D)              # 512 rows
        krows = slice(C + h0 * D, C + (h0 + HPC) * D)
        vrows = slice(2 * C + h0 * D, 2 * C + (h0 + HPC) * D)
        wqk = np.concatenate([w_attn[qrows], w_attn[krows]], axis=0)  # [1024, C]
        bqk_np = np.concatenate([b_attn[qrows], b_attn[krows]])       # [1024]
        cst = np.concatenate([dmask,
                              bqk_np.reshape(8, P).T.astype(np.float32)],
                             axis=1)                                  # [128, 136]
        in_maps.append({
            "xT": np.ascontiguousarray(x[b].T).astype(BF16NP),         # [C, T]
            "wqkT": np.ascontiguousarray(wqk.T).astype(BF16NP),        # [C, 1024]
            "wvT": np.ascontiguousarray(w_attn[vrows].T).astype(BF16NP),
            "wpT": np.ascontiguousarray(
                w_proj[:, h0 * D:(h0 + HPC) * D].T).astype(BF16NP),
            "csT": csT,
            "cstT": np.ascontiguousarray(cst),
            "rmat": rmat,
            "bv": np.ascontiguousarray(b_attn[vrows].reshape(1, 512)
                                       .astype(np.float32)),
        })
    return in_maps


def kernel(x, w_attn, b_attn, w_proj, b_proj, cos, sin):
    global _NC_CACHE
    x = np.asarray(x, np.float32)
    w_attn = np.asarray(w_attn, np.float32)
    b_attn = np.asarray(b_attn, np.float32)
    w_proj = np.asarray(w_proj, np.float32)
    b_proj = np.asarray(b_proj, np.float32)
    cos = np.asarray(cos, np.float32)
    sin = np.asarray(sin, np.float32)

    if _NC_CACHE is None:
        _NC_CACHE = build_nc()
    nc = _NC_CACHE
    in_maps = _host_inputs(x, w_attn, b_attn, w_proj, cos, sin)
    res = run_bass_kernel_spmd(nc, in_maps, core_ids=list(range(NCORES)))
    parts = [np.asarray(res.results[i]["outp"], np.float32)
             for i in range(NCORES)]
    out = np.empty((B, T, C), np.float32)
    for b in range(B):
        out[b] = parts[2 * b] + parts[2 * b + 1] + b_proj
    return out
